# revision 13
# baseline (speedup 1.0000x reference)
"""Slot-attention kernel for Trainium2, SPMD over 8 NeuronCores.

Math (reference, restructured):
  Q~ = queries @ Wk / sqrt(D)           [N, D]     (host, f32)
  c  = queries @ bk / sqrt(D)           [N]        (host, f32)
  logits[b,n,s] = x[b,s,:]. Q~[n,:] + c[n]
  softmax over n (slots), multiplicative mask, take slot IDX=3:
    w[b,s]   = exp(l[b,3,s])*m'[b,s,3] / sum_n exp(l[b,n,s])*m'[b,s,n]
    with m'[b,s,n] = mask[b,n,s] * exp(c[n]-c[3])   (c folds into the mask)
  attention[b,s] = w[b,s] / sum_s w[b,s]
  out[b,:] = (sum_s attention[b,s] * x[b,s,:]) @ Wv.T + bv   (since sum attention = 1)

Device layout (per core: 2 batches, S=8192 positions each, chunks of 512):
  logits tile [s=128p, n=64] = sum_h inT8[dh,s-128].T @ qt8[dh,n]  (fp8, PSUM f32)
  exp on ACT (scale=1/64 undoes the fp8 range scaling of qt8)
  DVE: masked = exp*mask', den = reduce_sum_n, w = masked[:,3]*recip(den)
  wsum: accT[d-half,1] += nat8[s,dh].T @ w8[s,1]  (fp8, w scaled x1024)
  epilogue: total via ones.T@colsum, out = (accT * inv/1024) @ WvT + bv
"""

import numpy as np
import ml_dtypes
from contextlib import ExitStack

import concourse.bass as bass
import concourse.bacc as bacc
import concourse.tile as tile
from concourse import mybir
from concourse.bass_utils import run_bass_kernel_spmd

B, S, D, N = 16, 8192, 256, 64
IDX = 3
NCORES = 8
BPC = B // NCORES          # batches per core
CH = 512                   # positions per chunk
T = CH // 128              # sub-tiles per chunk
NCH = S // CH              # chunks per batch
NT = S // 128              # 128-tiles per batch (64)
Q8SCALE = 64.0             # qt stored as Q~T * 64 (fp8 range), undone in exp scale
W8SCALE = 1024.0           # w stored as w * 1024 (fp8 range), undone in acc scale

FP8 = mybir.dt.float8e4
BF16 = mybir.dt.bfloat16
F32 = mybir.dt.float32

np_fp8 = ml_dtypes.float8_e4m3
np_bf16 = ml_dtypes.bfloat16


def _build_nc():
    nc = bacc.Bacc("TRN2", target_bir_lowering=False, debug=False)
    # inputs are host-pretiled: one contiguous [128, X] block per 512-pos chunk
    natp = nc.declare_dram_parameter("nat", [BPC, NCH, 128, T * D], BF16, isOutput=False)
    intp = nc.declare_dram_parameter("inT", [BPC, NCH, 128, 2 * CH], FP8, isOutput=False)
    mskp = nc.declare_dram_parameter("msk", [BPC, NCH, 128, T * N], BF16, isOutput=False)
    qtp = nc.declare_dram_parameter("qt", [D, N], FP8, isOutput=False)
    wvtp = nc.declare_dram_parameter("wvt", [D, D], F32, isOutput=False)
    bvp = nc.declare_dram_parameter("bv", [1, D], F32, isOutput=False)
    outv = nc.declare_dram_parameter("outv", [BPC, D], F32, isOutput=True)
    atto = nc.declare_dram_parameter("att", [BPC, 128, NT], F32, isOutput=True)
    accd = nc.declare_dram_parameter("accd", [BPC, 128, 2], F32, isOutput=True)

    with tile.TileContext(nc) as tc, ExitStack() as ctx:
        consts = ctx.enter_context(tc.tile_pool(name="consts", bufs=1))
        loads = ctx.enter_context(tc.tile_pool(name="loads", bufs=3))
        work = ctx.enter_context(tc.tile_pool(name="work", bufs=3))
        wbuf = ctx.enter_context(tc.tile_pool(name="wbuf", bufs=2))
        psum_l = ctx.enter_context(tc.tile_pool(name="psl", bufs=3, space="PSUM"))
        psum_a = ctx.enter_context(tc.tile_pool(name="psa", bufs=2, space="PSUM"))
        psum_e = ctx.enter_context(tc.tile_pool(name="pse", bufs=2, space="PSUM"))

        # constants
        qt_sb = consts.tile([128, 2, N], FP8)
        nc.sync.dma_start(out=qt_sb, in_=qtp[:].rearrange("(h p) n -> p h n", p=128))
        wvt_sb = consts.tile([128, 2, D], F32)
        nc.sync.dma_start(out=wvt_sb, in_=wvtp[:].rearrange("(h p) e -> p h e", p=128))
        bv_sb = consts.tile([1, D], F32)
        nc.sync.dma_start(out=bv_sb, in_=bvp[:])
        ones_col = consts.tile([128, 1], F32)
        nc.vector.memset(ones_col, 1.0)
        ones_row = consts.tile([1, 128], F32)
        nc.vector.memset(ones_row, 1.0)

        for b in range(BPC):
            w_all = wbuf.tile([128, NT], F32, tag="w_all")
            acc = psum_a.tile([128, 2], F32, tag="acc")
            for c in range(NCH):
                nat = loads.tile([128, T, D], BF16, tag="nat")
                nc.gpsimd.dma_start(
                    out=nat, in_=natp[b, c].rearrange("p (t d) -> p t d", t=T)
                )
                inT = loads.tile([128, 2, CH], FP8, tag="inT")
                nc.gpsimd.dma_start(
                    out=inT, in_=intp[b, c].rearrange("p (h s) -> p h s", h=2)
                )
                msk = loads.tile([128, T, N], BF16, tag="msk")
                nc.gpsimd.dma_start(
                    out=msk, in_=mskp[b, c].rearrange("p (t n) -> p t n", t=T)
                )

                # logits for the whole chunk into one PSUM bank [128, T*64]
                pl = psum_l.tile([128, T, N], F32, tag="pl")
                for t in range(T):
                    for h in range(2):
                        nc.tensor.matmul(
                            pl[:, t, :],
                            lhsT=inT[:, h, t * 128 : (t + 1) * 128],
                            rhs=qt_sb[:, h, :],
                            start=(h == 0),
                            stop=(h == 1),
                        )
                # exp (undo Q8SCALE), PSUM -> SBUF bf16
                exps = work.tile([128, T, N], BF16, tag="exps")
                nc.scalar.activation(
                    out=exps,
                    in_=pl,
                    func=mybir.ActivationFunctionType.Exp,
                    scale=1.0 / Q8SCALE,
                )
                # masked exp, denominator, w
                mes = work.tile([128, T, N], BF16, tag="mes")
                nc.vector.tensor_mul(mes, exps, msk)
                den = work.tile([128, T], F32, tag="den")
                nc.vector.reduce_sum(out=den, in_=mes, axis=mybir.AxisListType.X)
                iden = work.tile([128, T], F32, tag="iden")
                nc.vector.reciprocal(iden, den)
                wf = w_all[:, c * T : (c + 1) * T]
                nc.vector.tensor_mul(wf, mes[:, :, IDX], iden)
                w8 = work.tile([128, T], BF16, tag="w8")
                nc.vector.tensor_scalar_mul(w8, wf, W8SCALE)

                # weighted sum of x: accT[d-half, 1] over the whole batch
                for t in range(T):
                    for h in range(2):
                        # start only on the very first matmul: start=True clears
                        # the whole bank's has_written bits, so a second start
                        # (h=1) would wipe h=0's just-set bits and the next
                        # h=0 matmul would overwrite instead of accumulate.
                        nc.tensor.matmul(
                            acc[:, h : h + 1],
                            lhsT=nat[:, t, h * 128 : (h + 1) * 128],
                            rhs=w8[:, t : t + 1],
                            start=(c == 0 and t == 0 and h == 0),
                            stop=(c == NCH - 1 and t == T - 1 and h == 1),
                        )

            # ---- batch epilogue ----
            colsum = work.tile([128, 1], F32, tag="colsum")
            nc.vector.reduce_sum(out=colsum, in_=w_all, axis=mybir.AxisListType.X)
            tot = psum_e.tile([1, 1], F32, tag="eps")
            nc.tensor.matmul(tot, lhsT=ones_col, rhs=colsum, start=True, stop=True)
            inv = work.tile([1, 1], F32, tag="inv")
            nc.vector.reciprocal(inv, tot)
            invb_ps = psum_e.tile([128, 1], F32, tag="eps")
            nc.tensor.matmul(invb_ps, lhsT=ones_row, rhs=inv, start=True, stop=True)
            invb = work.tile([128, 1], F32, tag="invb")
            nc.vector.tensor_copy(invb, invb_ps)

            # attention output, [p, j] layout (host reorders)
            att_sb = wbuf.tile([128, NT], F32, tag="att_sb")
            nc.vector.tensor_scalar_mul(att_sb, w_all, invb)
            nc.sync.dma_start(out=atto[b], in_=att_sb)

            # out = (accT * inv / W8SCALE) @ WvT + bv
            accs = work.tile([128, 2], F32, tag="accs")
            nc.vector.tensor_scalar(
                accs,
                acc,
                scalar1=invb,
                scalar2=1.0 / W8SCALE,
                op0=mybir.AluOpType.mult,
                op1=mybir.AluOpType.mult,
            )
            nc.sync.dma_start(out=accd[b], in_=accs)
            pout = psum_e.tile([1, D], F32, tag="eps")
            for h in range(2):
                nc.tensor.matmul(
                    pout,
                    lhsT=accs[:, h : h + 1],
                    rhs=wvt_sb[:, h, :],
                    start=(h == 0),
                    stop=(h == 1),
                )
            outv_sb = work.tile([1, D], F32, tag="outv_sb")
            nc.vector.tensor_add(outv_sb, pout, bv_sb)
            nc.sync.dma_start(out=outv[b : b + 1, :], in_=outv_sb)

    nc.finalize()
    return nc


def _prep(input_embedding, mask, Wv, bv, Wk, bk, queries):
    x = np.asarray(input_embedding, dtype=np.float32)
    mask = np.asarray(mask)
    Wv = np.asarray(Wv, dtype=np.float32)
    bv = np.asarray(bv, dtype=np.float32)
    Wk = np.asarray(Wk, dtype=np.float32)
    bk = np.asarray(bk, dtype=np.float32)
    queries = np.asarray(queries, dtype=np.float32)

    qt = (queries @ Wk) / np.sqrt(D).astype(np.float32)      # [N, D] = Q~
    c = (queries @ bk) / np.sqrt(D).astype(np.float32)       # [N]
    cscale = np.exp(c - c[IDX]).astype(np.float32)           # [N]

    # pretile: chunk c covers positions [c*CH, (c+1)*CH); within a chunk,
    # SBUF partition p / sub-tile t holds position c*CH + t*128 + p.
    # nat[b, c, p, t*D+d] = x[b, c*CH + t*128 + p, d]
    nat8 = np.ascontiguousarray(
        x.reshape(B, NCH, T, 128, D).transpose(0, 1, 3, 2, 4).reshape(B, NCH, 128, T * D)
    ).astype(np_bf16)
    # inT[b, c, p, h*CH+s] = x[b, c*CH + s, 128*h + p]
    xT = x.transpose(0, 2, 1)  # [B, D, S]
    inT8 = np.ascontiguousarray(
        xT.reshape(B, 2, 128, NCH, CH).transpose(0, 3, 2, 1, 4).reshape(B, NCH, 128, 2 * CH)
    ).astype(np_fp8)
    # msk[b, c, p, t*N+n] = mask[b, n, c*CH + t*128 + p] * cscale[n]
    mT = mask.transpose(0, 2, 1).astype(np.float32) * cscale[None, None, :]  # [B, S, N]
    mskT = np.ascontiguousarray(
        mT.reshape(B, NCH, T, 128, N).transpose(0, 1, 3, 2, 4).reshape(B, NCH, 128, T * N)
    ).astype(np_bf16)
    qt8 = np.ascontiguousarray((qt * Q8SCALE).T).astype(np_fp8)  # [D, N]
    wvt = np.ascontiguousarray(Wv.T)                         # [D, D]
    return nat8, inT8, mskT, qt8, wvt, bv.reshape(1, D)


_CACHE = {}


def _run(inputs, trace=False):
    nat8, inT8, mskT, qt8, wvt, bvr = _prep(**inputs)
    if "nc" not in _CACHE:
        _CACHE["nc"] = _build_nc()
    nc = _CACHE["nc"]
    in_maps = []
    for i in range(NCORES):
        sl = slice(i * BPC, (i + 1) * BPC)
        in_maps.append(
            {
                "nat": nat8[sl],
                "inT": inT8[sl],
                "msk": mskT[sl],
                "qt": qt8,
                "wvt": wvt,
                "bv": bvr,
            }
        )
    res = run_bass_kernel_spmd(nc, in_maps, core_ids=list(range(NCORES)), trace=trace)
    outs = []
    atts = []
    for i in range(NCORES):
        outs.append(np.asarray(res.results[i]["outv"], dtype=np.float32))
        _CACHE.setdefault("accd", []).append(np.asarray(res.results[i]["accd"], dtype=np.float32))
        a = np.asarray(res.results[i]["att"], dtype=np.float32)  # [BPC, 128, NT]
        atts.append(a.transpose(0, 2, 1).reshape(BPC, S))        # s = j*128 + p
    out = np.concatenate(outs, axis=0).reshape(B, 1, D)
    attention = np.concatenate(atts, axis=0).reshape(B, 1, S)
    return (out, attention), res


def kernel(**inputs):
    (out, attention), _ = _run(inputs, trace=False)
    return out, attention


# revision 14
# speedup vs baseline: 1.2192x; 1.2192x over previous
"""Slot-attention kernel for Trainium2, SPMD over 8 NeuronCores.

Math (reference, restructured):
  Q~ = queries @ Wk / sqrt(D)           [N, D]     (host, f32)
  c  = queries @ bk / sqrt(D)           [N]        (host, f32)
  logits[b,n,s] = x[b,s,:]. Q~[n,:] + c[n]
  softmax over n (slots), multiplicative mask, take slot IDX=3:
    w[b,s]   = exp(l[b,3,s])*m'[b,s,3] / sum_n exp(l[b,n,s])*m'[b,s,n]
    with m'[b,s,n] = mask[b,n,s] * exp(c[n]-c[3])   (c folds into the mask)
  attention[b,s] = w[b,s] / sum_s w[b,s]
  out[b,:] = (sum_s attention[b,s] * x[b,s,:]) @ Wv.T + bv   (since sum attention = 1)

Device layout (per core: 2 batches, S=8192 positions each, chunks of 512):
  logits tile [s=128p, n=64] = sum_h inT8[dh,s-128].T @ qt8[dh,n]  (fp8, PSUM f32)
  exp on ACT (scale=1/64 undoes the fp8 range scaling of qt8)
  DVE: masked = exp*mask', den = reduce_sum_n, w = masked[:,3]*recip(den)
  wsum: accT[d-half,1] += nat8[s,dh].T @ w8[s,1]  (fp8, w scaled x1024)
  epilogue: total via ones.T@colsum, out = (accT * inv/1024) @ WvT + bv
"""

import numpy as np
import ml_dtypes
from contextlib import ExitStack

import concourse.bass as bass
import concourse.bacc as bacc
import concourse.tile as tile
from concourse import mybir
from concourse.bass_utils import run_bass_kernel_spmd

B, S, D, N = 16, 8192, 256, 64
IDX = 3
NCORES = 8
BPC = B // NCORES          # batches per core
CH = 1024                  # positions per chunk
T = CH // 128              # sub-tiles per chunk
NCH = S // CH              # chunks per batch
NT = S // 128              # 128-tiles per batch (64)
Q8SCALE = 64.0             # qt stored as Q~T * 64 (fp8 range), undone in exp scale
W8SCALE = 1024.0           # w stored as w * 1024 (fp8 range), undone in acc scale

FP8 = mybir.dt.float8e4
BF16 = mybir.dt.bfloat16
F32 = mybir.dt.float32

np_fp8 = ml_dtypes.float8_e4m3
np_bf16 = ml_dtypes.bfloat16


def _wsum(nc, acc, nat, w8, first, last):
    # acc[0, :] += sum_t nat[:, t, :].T-contracted with w8[:, t]
    for t in range(T):
        nc.tensor.matmul(
            acc,
            lhsT=w8[:, t : t + 1],
            rhs=nat[:, t, :],
            start=(first and t == 0),
            stop=(last and t == T - 1),
        )


def _build_nc():
    nc = bacc.Bacc("TRN2", target_bir_lowering=False, debug=False)
    # inputs are host-pretiled: one contiguous [128, X] block per 512-pos chunk
    natp = nc.declare_dram_parameter("nat", [BPC, NCH, 128, T * D], BF16, isOutput=False)
    intp = nc.declare_dram_parameter("inT", [BPC, NCH, 128, 2 * CH], FP8, isOutput=False)
    mskp = nc.declare_dram_parameter("msk", [BPC, NCH, 128, T * N], BF16, isOutput=False)
    qtp = nc.declare_dram_parameter("qt", [D, N], FP8, isOutput=False)
    wvtp = nc.declare_dram_parameter("wvt", [D, D], F32, isOutput=False)
    bvp = nc.declare_dram_parameter("bv", [1, D], F32, isOutput=False)
    outv = nc.declare_dram_parameter("outv", [BPC, D], F32, isOutput=True)
    atto = nc.declare_dram_parameter("att", [BPC, 128, NT], F32, isOutput=True)

    with tile.TileContext(nc) as tc, ExitStack() as ctx:
        consts = ctx.enter_context(tc.tile_pool(name="consts", bufs=1))
        loads = ctx.enter_context(tc.tile_pool(name="loads", bufs=4))
        work = ctx.enter_context(tc.tile_pool(name="work", bufs=4))
        wbuf = ctx.enter_context(tc.tile_pool(name="wbuf", bufs=2))
        psum_l = ctx.enter_context(tc.tile_pool(name="psl", bufs=3, space="PSUM"))
        psum_a = ctx.enter_context(tc.tile_pool(name="psa", bufs=2, space="PSUM"))
        psum_e = ctx.enter_context(tc.tile_pool(name="pse", bufs=2, space="PSUM"))

        # constants
        qt_sb = consts.tile([128, 2, N], FP8)
        nc.sync.dma_start(out=qt_sb, in_=qtp[:].rearrange("(h p) n -> p h n", p=128))
        wvt_sb = consts.tile([128, 2, D], F32)
        nc.sync.dma_start(out=wvt_sb, in_=wvtp[:].rearrange("(h p) e -> p h e", p=128))
        bv_sb = consts.tile([1, D], F32)
        nc.sync.dma_start(out=bv_sb, in_=bvp[:])
        ones_col = consts.tile([128, 1], F32)
        nc.vector.memset(ones_col, 1.0)
        ones_row = consts.tile([1, 128], F32)
        nc.vector.memset(ones_row, 1.0)

        for b in range(BPC):
            w_all = wbuf.tile([128, NT], F32, tag="w_all")
            acc = psum_a.tile([1, D], F32, tag="acc")
            w8s = []
            nats = []
            for c in range(NCH):
                nat = loads.tile([128, T, D], BF16, tag="nat")
                nc.sync.dma_start(
                    out=nat, in_=natp[b, c].rearrange("p (t d) -> p t d", t=T)
                )
                inT = loads.tile([128, 2, CH], FP8, tag="inT")
                nc.sync.dma_start(
                    out=inT, in_=intp[b, c].rearrange("p (h s) -> p h s", h=2)
                )
                msk = loads.tile([128, T, N], BF16, tag="msk")
                nc.sync.dma_start(
                    out=msk, in_=mskp[b, c].rearrange("p (t n) -> p t n", t=T)
                )

                # logits for the whole chunk into one PSUM bank [128, T*64]
                pl = psum_l.tile([128, T, N], F32, tag="pl")
                for t in range(T):
                    for h in range(2):
                        nc.tensor.matmul(
                            pl[:, t, :],
                            lhsT=inT[:, h, t * 128 : (t + 1) * 128],
                            rhs=qt_sb[:, h, :],
                            start=(h == 0),
                            stop=(h == 1),
                        )
                # exp (undo Q8SCALE), PSUM -> SBUF bf16
                exps = work.tile([128, T, N], BF16, tag="exps")
                nc.scalar.activation(
                    out=exps,
                    in_=pl,
                    func=mybir.ActivationFunctionType.Exp,
                    scale=1.0 / Q8SCALE,
                )
                # masked exp, denominator, w
                mes = work.tile([128, T, N], BF16, tag="mes")
                nc.vector.tensor_mul(mes, exps, msk)
                den = work.tile([128, T], F32, tag="den")
                nc.vector.reduce_sum(out=den, in_=mes, axis=mybir.AxisListType.X)
                iden = work.tile([128, T], F32, tag="iden")
                nc.vector.reciprocal(iden, den)
                wf = w_all[:, c * T : (c + 1) * T]
                nc.vector.tensor_mul(wf, mes[:, :, IDX], iden)
                w8 = work.tile([128, T], BF16, tag="w8")
                nc.vector.tensor_scalar_mul(w8, wf, W8SCALE)
                w8s.append(w8)
                nats.append(nat)

                # weighted sum for the PREVIOUS chunk (software pipeline: keeps
                # PE from stalling on this chunk's softmax chain)
                if c > 0:
                    _wsum(nc, acc, nats[c - 1], w8s[c - 1], first=(c == 1), last=False)
            _wsum(nc, acc, nats[NCH - 1], w8s[NCH - 1], first=(NCH == 1), last=True)

            # ---- batch epilogue ----
            colsum = work.tile([128, 1], F32, tag="colsum")
            nc.vector.reduce_sum(out=colsum, in_=w_all, axis=mybir.AxisListType.X)
            tot = psum_e.tile([1, 1], F32, tag="eps")
            nc.tensor.matmul(tot, lhsT=ones_col, rhs=colsum, start=True, stop=True)
            inv = work.tile([1, 1], F32, tag="inv")
            nc.vector.reciprocal(inv, tot)
            invb_ps = psum_e.tile([128, 1], F32, tag="eps")
            nc.tensor.matmul(invb_ps, lhsT=ones_row, rhs=inv, start=True, stop=True)
            invb = work.tile([128, 1], F32, tag="invb")
            nc.vector.tensor_copy(invb, invb_ps)

            # attention output, [p, j] layout (host reorders)
            att_sb = wbuf.tile([128, NT], F32, tag="att_sb")
            nc.vector.tensor_scalar_mul(att_sb, w_all, invb)
            nc.sync.dma_start(out=atto[b], in_=att_sb)

            # accs_row = acc * inv / W8SCALE   [1, D]
            accs = work.tile([1, D], F32, tag="accs")
            nc.vector.tensor_scalar(
                accs,
                acc,
                scalar1=inv,
                scalar2=1.0 / W8SCALE,
                op0=mybir.AluOpType.mult,
                op1=mybir.AluOpType.mult,
            )
            # transpose accs halves to columns, then project: out = accsT @ WvT + bv
            accT = work.tile([128, 2], F32, tag="accT")
            for h in range(2):
                tp = psum_e.tile([128, 1], F32, tag="eps")
                nc.tensor.matmul(
                    tp,
                    lhsT=accs[:, h * 128 : (h + 1) * 128],
                    rhs=ones_row[0:1, 0:1],
                    start=True,
                    stop=True,
                )
                nc.vector.tensor_copy(accT[:, h : h + 1], tp)
            pout = psum_e.tile([1, D], F32, tag="eps")
            for h in range(2):
                nc.tensor.matmul(
                    pout,
                    lhsT=accT[:, h : h + 1],
                    rhs=wvt_sb[:, h, :],
                    start=(h == 0),
                    stop=(h == 1),
                )
            outv_sb = work.tile([1, D], F32, tag="outv_sb")
            nc.vector.tensor_add(outv_sb, pout, bv_sb)
            nc.sync.dma_start(out=outv[b : b + 1, :], in_=outv_sb)

    nc.finalize()
    return nc


def _prep(input_embedding, mask, Wv, bv, Wk, bk, queries):
    x = np.asarray(input_embedding, dtype=np.float32)
    mask = np.asarray(mask)
    Wv = np.asarray(Wv, dtype=np.float32)
    bv = np.asarray(bv, dtype=np.float32)
    Wk = np.asarray(Wk, dtype=np.float32)
    bk = np.asarray(bk, dtype=np.float32)
    queries = np.asarray(queries, dtype=np.float32)

    qt = (queries @ Wk) / np.sqrt(D).astype(np.float32)      # [N, D] = Q~
    c = (queries @ bk) / np.sqrt(D).astype(np.float32)       # [N]
    cscale = np.exp(c - c[IDX]).astype(np.float32)           # [N]

    # pretile: chunk c covers positions [c*CH, (c+1)*CH); within a chunk,
    # SBUF partition p / sub-tile t holds position c*CH + t*128 + p.
    # nat[b, c, p, t*D+d] = x[b, c*CH + t*128 + p, d]
    nat8 = np.ascontiguousarray(
        x.reshape(B, NCH, T, 128, D).transpose(0, 1, 3, 2, 4).reshape(B, NCH, 128, T * D)
    ).astype(np_bf16)
    # inT[b, c, p, h*CH+s] = x[b, c*CH + s, 128*h + p]
    xT = x.transpose(0, 2, 1)  # [B, D, S]
    inT8 = np.ascontiguousarray(
        xT.reshape(B, 2, 128, NCH, CH).transpose(0, 3, 2, 1, 4).reshape(B, NCH, 128, 2 * CH)
    ).astype(np_fp8)
    # msk[b, c, p, t*N+n] = mask[b, n, c*CH + t*128 + p] * cscale[n]
    mT = mask.transpose(0, 2, 1).astype(np.float32) * cscale[None, None, :]  # [B, S, N]
    mskT = np.ascontiguousarray(
        mT.reshape(B, NCH, T, 128, N).transpose(0, 1, 3, 2, 4).reshape(B, NCH, 128, T * N)
    ).astype(np_bf16)
    qt8 = np.ascontiguousarray((qt * Q8SCALE).T).astype(np_fp8)  # [D, N]
    wvt = np.ascontiguousarray(Wv.T)                         # [D, D]
    return nat8, inT8, mskT, qt8, wvt, bv.reshape(1, D)


_CACHE = {}


def _run(inputs, trace=False):
    nat8, inT8, mskT, qt8, wvt, bvr = _prep(**inputs)
    if "nc" not in _CACHE:
        _CACHE["nc"] = _build_nc()
    nc = _CACHE["nc"]
    in_maps = []
    for i in range(NCORES):
        sl = slice(i * BPC, (i + 1) * BPC)
        in_maps.append(
            {
                "nat": nat8[sl],
                "inT": inT8[sl],
                "msk": mskT[sl],
                "qt": qt8,
                "wvt": wvt,
                "bv": bvr,
            }
        )
    res = run_bass_kernel_spmd(nc, in_maps, core_ids=list(range(NCORES)), trace=trace)
    outs = []
    atts = []
    for i in range(NCORES):
        outs.append(np.asarray(res.results[i]["outv"], dtype=np.float32))
        a = np.asarray(res.results[i]["att"], dtype=np.float32)  # [BPC, 128, NT]
        atts.append(a.transpose(0, 2, 1).reshape(BPC, S))        # s = j*128 + p
    out = np.concatenate(outs, axis=0).reshape(B, 1, D)
    attention = np.concatenate(atts, axis=0).reshape(B, 1, S)
    return (out, attention), res


def kernel(**inputs):
    (out, attention), _ = _run(inputs, trace=False)
    return out, attention


# revision 15
# speedup vs baseline: 1.2826x; 1.0520x over previous
"""Slot-attention kernel for Trainium2, SPMD over 8 NeuronCores.

Math (reference, restructured):
  Q~ = queries @ Wk / sqrt(D)           [N, D]     (host, f32)
  c  = queries @ bk / sqrt(D)           [N]        (host, f32)
  logits[b,n,s] = x[b,s,:]. Q~[n,:] + c[n]
  softmax over n (slots), multiplicative mask, take slot IDX=3:
    w[b,s]   = exp(l[b,3,s])*m'[b,s,3] / sum_n exp(l[b,n,s])*m'[b,s,n]
    with m'[b,s,n] = mask[b,n,s] * exp(c[n]-c[3])   (c folds into the mask)
  attention[b,s] = w[b,s] / sum_s w[b,s]
  out[b,:] = (sum_s attention[b,s] * x[b,s,:]) @ Wv.T + bv   (since sum attention = 1)

Device layout (per core: 2 batches, S=8192 positions each, chunks of 512):
  logits tile [s=128p, n=64] = sum_h inT8[dh,s-128].T @ qt8[dh,n]  (fp8, PSUM f32)
  exp on ACT (scale=1/64 undoes the fp8 range scaling of qt8)
  DVE: masked = exp*mask', den = reduce_sum_n, w = masked[:,3]*recip(den)
  wsum: accT[d-half,1] += nat8[s,dh].T @ w8[s,1]  (fp8, w scaled x1024)
  epilogue: total via ones.T@colsum, out = (accT * inv/1024) @ WvT + bv
"""

import numpy as np
import ml_dtypes
from contextlib import ExitStack

import concourse.bass as bass
import concourse.bacc as bacc
import concourse.tile as tile
from concourse import mybir
from concourse.bass_utils import run_bass_kernel_spmd

B, S, D, N = 16, 8192, 256, 64
IDX = 3
NCORES = 8
BPC = B // NCORES          # batches per core
CH = 1024                  # positions per chunk
T = CH // 128              # sub-tiles per chunk
NCH = S // CH              # chunks per batch
NT = S // 128              # 128-tiles per batch (64)
Q8SCALE = 64.0             # qt stored as Q~T * 64 (fp8 range), undone in exp scale
W8SCALE = 1024.0           # w stored as w * 1024 (fp8 range), undone in acc scale

FP8 = mybir.dt.float8e4
BF16 = mybir.dt.bfloat16
F32 = mybir.dt.float32

np_fp8 = ml_dtypes.float8_e4m3
np_bf16 = ml_dtypes.bfloat16


def _wsum(nc, acc, nat, w8, first, last):
    # acc[0, :] += sum_t nat[:, t, :].T-contracted with w8[:, t]
    for t in range(T):
        nc.tensor.matmul(
            acc,
            lhsT=w8[:, t : t + 1],
            rhs=nat[:, t, :],
            start=(first and t == 0),
            stop=(last and t == T - 1),
        )


def _build_nc():
    nc = bacc.Bacc("TRN2", target_bir_lowering=False, debug=False)
    # inputs are host-pretiled: one contiguous [128, X] block per 512-pos chunk
    natp = nc.declare_dram_parameter("nat", [BPC, NCH, 128, T * D], BF16, isOutput=False)
    intp = nc.declare_dram_parameter("inT", [BPC, NCH, 128, 2 * CH], FP8, isOutput=False)
    mskp = nc.declare_dram_parameter("msk", [BPC, NCH, 128, T * N], BF16, isOutput=False)
    qtp = nc.declare_dram_parameter("qt", [D, N], FP8, isOutput=False)
    wvtp = nc.declare_dram_parameter("wvt", [D, D], F32, isOutput=False)
    bvp = nc.declare_dram_parameter("bv", [1, D], F32, isOutput=False)
    outv = nc.declare_dram_parameter("outv", [BPC, D], F32, isOutput=True)
    atto = nc.declare_dram_parameter("att", [BPC, 128, NT], F32, isOutput=True)

    with tile.TileContext(nc) as tc, ExitStack() as ctx:
        consts = ctx.enter_context(tc.tile_pool(name="consts", bufs=1))
        loads = ctx.enter_context(tc.tile_pool(name="loads", bufs=6))
        work = ctx.enter_context(tc.tile_pool(name="work", bufs=6))
        wbuf = ctx.enter_context(tc.tile_pool(name="wbuf", bufs=2))
        psum_l = ctx.enter_context(tc.tile_pool(name="psl", bufs=4, space="PSUM"))
        psum_a = ctx.enter_context(tc.tile_pool(name="psa", bufs=2, space="PSUM"))
        psum_e = ctx.enter_context(tc.tile_pool(name="pse", bufs=2, space="PSUM"))

        # constants
        qt_sb = consts.tile([128, 2, N], FP8)
        nc.sync.dma_start(out=qt_sb, in_=qtp[:].rearrange("(h p) n -> p h n", p=128))
        wvt_sb = consts.tile([128, 2, D], F32)
        nc.sync.dma_start(out=wvt_sb, in_=wvtp[:].rearrange("(h p) e -> p h e", p=128))
        bv_sb = consts.tile([1, D], F32)
        nc.sync.dma_start(out=bv_sb, in_=bvp[:])
        ones_col = consts.tile([128, 1], F32)
        nc.vector.memset(ones_col, 1.0)
        ones_row = consts.tile([1, 128], F32)
        nc.vector.memset(ones_row, 1.0)

        for b in range(BPC):
            w_all = wbuf.tile([128, NT], F32, tag="w_all")
            acc = psum_a.tile([1, D], F32, tag="acc")
            w8s = []
            nats = []
            for c in range(NCH):
                nat = loads.tile([128, T, D], BF16, tag="nat")
                nc.sync.dma_start(
                    out=nat, in_=natp[b, c].rearrange("p (t d) -> p t d", t=T)
                )
                inT = loads.tile([128, 2, CH], FP8, tag="inT")
                nc.sync.dma_start(
                    out=inT, in_=intp[b, c].rearrange("p (h s) -> p h s", h=2)
                )
                msk = loads.tile([128, T, N], BF16, tag="msk")
                nc.sync.dma_start(
                    out=msk, in_=mskp[b, c].rearrange("p (t n) -> p t n", t=T)
                )

                # logits for the whole chunk into one PSUM bank [128, T*64]
                pl = psum_l.tile([128, T, N], F32, tag="pl")
                for t in range(T):
                    for h in range(2):
                        nc.tensor.matmul(
                            pl[:, t, :],
                            lhsT=inT[:, h, t * 128 : (t + 1) * 128],
                            rhs=qt_sb[:, h, :],
                            start=(h == 0),
                            stop=(h == 1),
                        )
                # exp (undo Q8SCALE), PSUM -> SBUF bf16
                exps = work.tile([128, T, N], BF16, tag="exps")
                nc.scalar.activation(
                    out=exps,
                    in_=pl,
                    func=mybir.ActivationFunctionType.Exp,
                    scale=1.0 / Q8SCALE,
                )
                # masked exp, denominator, w
                mes = work.tile([128, T, N], BF16, tag="mes")
                nc.vector.tensor_mul(mes, exps, msk)
                den = work.tile([128, T], F32, tag="den")
                nc.vector.reduce_sum(out=den, in_=mes, axis=mybir.AxisListType.X)
                iden = work.tile([128, T], F32, tag="iden")
                nc.vector.reciprocal(iden, den)
                wf = w_all[:, c * T : (c + 1) * T]
                nc.vector.tensor_mul(wf, mes[:, :, IDX], iden)
                w8 = work.tile([128, T], BF16, tag="w8")
                nc.vector.tensor_scalar_mul(w8, wf, W8SCALE)
                w8s.append(w8)
                nats.append(nat)

                # weighted sum lagged two chunks (software pipeline: gives the
                # softmax chain two chunk-periods before PE needs its w8)
                if c >= 2:
                    _wsum(nc, acc, nats[c - 2], w8s[c - 2], first=(c == 2), last=False)
            _wsum(nc, acc, nats[NCH - 2], w8s[NCH - 2], first=(NCH == 2), last=False)
            _wsum(nc, acc, nats[NCH - 1], w8s[NCH - 1], first=False, last=True)

            # ---- batch epilogue ----
            colsum = work.tile([128, 1], F32, tag="colsum")
            nc.vector.reduce_sum(out=colsum, in_=w_all, axis=mybir.AxisListType.X)
            tot = psum_e.tile([1, 1], F32, tag="eps")
            nc.tensor.matmul(tot, lhsT=ones_col, rhs=colsum, start=True, stop=True)
            inv = work.tile([1, 1], F32, tag="inv")
            nc.vector.reciprocal(inv, tot)
            invb_ps = psum_e.tile([128, 1], F32, tag="eps")
            nc.tensor.matmul(invb_ps, lhsT=ones_row, rhs=inv, start=True, stop=True)
            invb = work.tile([128, 1], F32, tag="invb")
            nc.vector.tensor_copy(invb, invb_ps)

            # attention output, [p, j] layout (host reorders)
            att_sb = wbuf.tile([128, NT], F32, tag="att_sb")
            nc.vector.tensor_scalar_mul(att_sb, w_all, invb)
            nc.sync.dma_start(out=atto[b], in_=att_sb)

            # accs_row = acc * inv / W8SCALE   [1, D]
            accs = work.tile([1, D], F32, tag="accs")
            nc.vector.tensor_scalar(
                accs,
                acc,
                scalar1=inv,
                scalar2=1.0 / W8SCALE,
                op0=mybir.AluOpType.mult,
                op1=mybir.AluOpType.mult,
            )
            # transpose accs halves to columns, then project: out = accsT @ WvT + bv
            accT = work.tile([128, 2], F32, tag="accT")
            for h in range(2):
                tp = psum_e.tile([128, 1], F32, tag="eps")
                nc.tensor.matmul(
                    tp,
                    lhsT=accs[:, h * 128 : (h + 1) * 128],
                    rhs=ones_row[0:1, 0:1],
                    start=True,
                    stop=True,
                )
                nc.vector.tensor_copy(accT[:, h : h + 1], tp)
            pout = psum_e.tile([1, D], F32, tag="eps")
            for h in range(2):
                nc.tensor.matmul(
                    pout,
                    lhsT=accT[:, h : h + 1],
                    rhs=wvt_sb[:, h, :],
                    start=(h == 0),
                    stop=(h == 1),
                )
            outv_sb = work.tile([1, D], F32, tag="outv_sb")
            nc.vector.tensor_add(outv_sb, pout, bv_sb)
            nc.sync.dma_start(out=outv[b : b + 1, :], in_=outv_sb)

    nc.finalize()
    return nc


def _prep(input_embedding, mask, Wv, bv, Wk, bk, queries):
    x = np.asarray(input_embedding, dtype=np.float32)
    mask = np.asarray(mask)
    Wv = np.asarray(Wv, dtype=np.float32)
    bv = np.asarray(bv, dtype=np.float32)
    Wk = np.asarray(Wk, dtype=np.float32)
    bk = np.asarray(bk, dtype=np.float32)
    queries = np.asarray(queries, dtype=np.float32)

    qt = (queries @ Wk) / np.sqrt(D).astype(np.float32)      # [N, D] = Q~
    c = (queries @ bk) / np.sqrt(D).astype(np.float32)       # [N]
    cscale = np.exp(c - c[IDX]).astype(np.float32)           # [N]

    # pretile: chunk c covers positions [c*CH, (c+1)*CH); within a chunk,
    # SBUF partition p / sub-tile t holds position c*CH + t*128 + p.
    # nat[b, c, p, t*D+d] = x[b, c*CH + t*128 + p, d]
    nat8 = np.ascontiguousarray(
        x.reshape(B, NCH, T, 128, D).transpose(0, 1, 3, 2, 4).reshape(B, NCH, 128, T * D)
    ).astype(np_bf16)
    # inT[b, c, p, h*CH+s] = x[b, c*CH + s, 128*h + p]
    xT = x.transpose(0, 2, 1)  # [B, D, S]
    inT8 = np.ascontiguousarray(
        xT.reshape(B, 2, 128, NCH, CH).transpose(0, 3, 2, 1, 4).reshape(B, NCH, 128, 2 * CH)
    ).astype(np_fp8)
    # msk[b, c, p, t*N+n] = mask[b, n, c*CH + t*128 + p] * cscale[n]
    mT = mask.transpose(0, 2, 1).astype(np.float32) * cscale[None, None, :]  # [B, S, N]
    mskT = np.ascontiguousarray(
        mT.reshape(B, NCH, T, 128, N).transpose(0, 1, 3, 2, 4).reshape(B, NCH, 128, T * N)
    ).astype(np_bf16)
    qt8 = np.ascontiguousarray((qt * Q8SCALE).T).astype(np_fp8)  # [D, N]
    wvt = np.ascontiguousarray(Wv.T)                         # [D, D]
    return nat8, inT8, mskT, qt8, wvt, bv.reshape(1, D)


_CACHE = {}


def _run(inputs, trace=False):
    nat8, inT8, mskT, qt8, wvt, bvr = _prep(**inputs)
    if "nc" not in _CACHE:
        _CACHE["nc"] = _build_nc()
    nc = _CACHE["nc"]
    in_maps = []
    for i in range(NCORES):
        sl = slice(i * BPC, (i + 1) * BPC)
        in_maps.append(
            {
                "nat": nat8[sl],
                "inT": inT8[sl],
                "msk": mskT[sl],
                "qt": qt8,
                "wvt": wvt,
                "bv": bvr,
            }
        )
    res = run_bass_kernel_spmd(nc, in_maps, core_ids=list(range(NCORES)), trace=trace)
    outs = []
    atts = []
    for i in range(NCORES):
        outs.append(np.asarray(res.results[i]["outv"], dtype=np.float32))
        a = np.asarray(res.results[i]["att"], dtype=np.float32)  # [BPC, 128, NT]
        atts.append(a.transpose(0, 2, 1).reshape(BPC, S))        # s = j*128 + p
    out = np.concatenate(outs, axis=0).reshape(B, 1, D)
    attention = np.concatenate(atts, axis=0).reshape(B, 1, S)
    return (out, attention), res


def kernel(**inputs):
    (out, attention), _ = _run(inputs, trace=False)
    return out, attention


# revision 18
# speedup vs baseline: 1.3671x; 1.0659x over previous
"""Slot-attention kernel for Trainium2, SPMD over 8 NeuronCores.

Math (reference, restructured):
  Q~ = queries @ Wk / sqrt(D)           [N, D]     (host, f32)
  c  = queries @ bk / sqrt(D)           [N]        (host, f32)
  logits[b,n,s] = x[b,s,:]. Q~[n,:] + c[n]
  softmax over n (slots), multiplicative mask, take slot IDX=3:
    w[b,s]   = exp(l[b,3,s])*m'[b,s,3] / sum_n exp(l[b,n,s])*m'[b,s,n]
    with m'[b,s,n] = mask[b,n,s] * exp(c[n]-c[3])   (c folds into the mask)
  attention[b,s] = w[b,s] / sum_s w[b,s]
  out[b,:] = (sum_s attention[b,s] * x[b,s,:]) @ Wv.T + bv   (since sum attention = 1)

Device layout (per core: 2 batches, S=8192 positions each, chunks of 512):
  logits tile [s=128p, n=64] = sum_h inT8[dh,s-128].T @ qt8[dh,n]  (fp8, PSUM f32)
  exp on ACT (scale=1/64 undoes the fp8 range scaling of qt8)
  DVE: masked = exp*mask', den = reduce_sum_n, w = masked[:,3]*recip(den)
  wsum: accT[d-half,1] += nat8[s,dh].T @ w8[s,1]  (fp8, w scaled x1024)
  epilogue: total via ones.T@colsum, out = (accT * inv/1024) @ WvT + bv
"""

import numpy as np
import ml_dtypes
from contextlib import ExitStack

import concourse.bass as bass
import concourse.bacc as bacc
import concourse.tile as tile
from concourse import mybir
from concourse.bass_utils import run_bass_kernel_spmd

B, S, D, N = 16, 8192, 256, 64
IDX = 3
NCORES = 8
BPC = B // NCORES          # batches per core
CH = 1024                  # positions per chunk
T = CH // 128              # sub-tiles per chunk
NCH = S // CH              # chunks per batch
NT = S // 128              # 128-tiles per batch (64)
Q8SCALE = 64.0             # qt stored as Q~T * 64 (fp8 range), undone in exp scale
W8SCALE = 1024.0           # w stored as w * 1024 (fp8 range), undone in acc scale

FP8 = mybir.dt.float8e4
BF16 = mybir.dt.bfloat16
F32 = mybir.dt.float32

np_fp8 = ml_dtypes.float8_e4m3
np_bf16 = ml_dtypes.bfloat16


def _wsum(nc, acc, nat, w8, first, last):
    # acc[0, :] += sum_t nat[:, t, :].T-contracted with w8[:, t]
    for t in range(T):
        nc.tensor.matmul(
            acc,
            lhsT=w8[:, t : t + 1],
            rhs=nat[:, t, :],
            start=(first and t == 0),
            stop=(last and t == T - 1),
        )


def _build_nc():
    nc = bacc.Bacc("TRN2", target_bir_lowering=False, debug=False)
    # inputs are host-pretiled: one contiguous [128, X] block per 512-pos chunk
    natp = nc.declare_dram_parameter("nat", [BPC, NCH, 128, T * D], BF16, isOutput=False)
    intp = nc.declare_dram_parameter("inT", [BPC, NCH, 128, 2 * CH], FP8, isOutput=False)
    mskp = nc.declare_dram_parameter("msk", [BPC, NCH, 128, T * N], BF16, isOutput=False)
    qtp = nc.declare_dram_parameter("qt", [D, N], FP8, isOutput=False)
    wvtp = nc.declare_dram_parameter("wvt", [D, D], F32, isOutput=False)
    bvp = nc.declare_dram_parameter("bv", [1, D], F32, isOutput=False)
    outv = nc.declare_dram_parameter("outv", [BPC, D], F32, isOutput=True)
    atto = nc.declare_dram_parameter("att", [BPC, 128, NT], F32, isOutput=True)

    with tile.TileContext(nc) as tc, ExitStack() as ctx:
        consts = ctx.enter_context(tc.tile_pool(name="consts", bufs=1))
        loads = ctx.enter_context(tc.tile_pool(name="loads", bufs=6))
        work = ctx.enter_context(tc.tile_pool(name="work", bufs=8))
        wbuf = ctx.enter_context(tc.tile_pool(name="wbuf", bufs=2))
        psum_l = ctx.enter_context(tc.tile_pool(name="psl", bufs=4, space="PSUM"))
        psum_a = ctx.enter_context(tc.tile_pool(name="psa", bufs=2, space="PSUM"))
        psum_e = ctx.enter_context(tc.tile_pool(name="pse", bufs=2, space="PSUM"))

        # constants
        qt_sb = consts.tile([128, 2, N], FP8)
        nc.sync.dma_start(out=qt_sb, in_=qtp[:].rearrange("(h p) n -> p h n", p=128))
        wvt_sb = consts.tile([128, 2, D], F32)
        nc.sync.dma_start(out=wvt_sb, in_=wvtp[:].rearrange("(h p) e -> p h e", p=128))
        bv_sb = consts.tile([1, D], F32)
        nc.sync.dma_start(out=bv_sb, in_=bvp[:])
        ones_col = consts.tile([128, 1], F32)
        nc.vector.memset(ones_col, 1.0)
        ones_row = consts.tile([1, 128], F32)
        nc.vector.memset(ones_row, 1.0)

        for b in range(BPC):
            w_all = wbuf.tile([128, NT], F32, tag="w_all")
            acc = psum_a.tile([1, D], F32, tag="acc")
            w8s = []
            nats = []
            for c in range(NCH):
                nat = loads.tile([128, T, D], BF16, tag="nat")
                nc.sync.dma_start(
                    out=nat, in_=natp[b, c].rearrange("p (t d) -> p t d", t=T)
                )
                inT = loads.tile([128, 2, CH], FP8, tag="inT")
                nc.sync.dma_start(
                    out=inT, in_=intp[b, c].rearrange("p (h s) -> p h s", h=2)
                )
                msk = loads.tile([128, T, N], BF16, tag="msk")
                nc.sync.dma_start(
                    out=msk, in_=mskp[b, c].rearrange("p (t n) -> p t n", t=T)
                )

                # logits for the whole chunk into one PSUM bank [128, T*64]
                pl = psum_l.tile([128, T, N], F32, tag="pl")
                for t in range(T):
                    for h in range(2):
                        nc.tensor.matmul(
                            pl[:, t, :],
                            lhsT=inT[:, h, t * 128 : (t + 1) * 128],
                            rhs=qt_sb[:, h, :],
                            start=(h == 0),
                            stop=(h == 1),
                        )
                # exp (undo Q8SCALE), PSUM -> SBUF bf16
                exps = work.tile([128, T, N], BF16, tag="exps")
                nc.scalar.activation(
                    out=exps,
                    in_=pl,
                    func=mybir.ActivationFunctionType.Exp,
                    scale=1.0 / Q8SCALE,
                )
                # masked exp, denominator, w
                mes = work.tile([128, T, N], BF16, tag="mes")
                nc.gpsimd.tensor_mul(mes, exps, msk)
                den = work.tile([128, T], F32, tag="den")
                nc.vector.reduce_sum(out=den, in_=mes, axis=mybir.AxisListType.X)
                iden = work.tile([128, T], F32, tag="iden")
                nc.vector.reciprocal(iden, den)
                wf = w_all[:, c * T : (c + 1) * T]
                nc.vector.tensor_mul(wf, mes[:, :, IDX], iden)
                w8 = work.tile([128, T], BF16, tag="w8")
                nc.vector.tensor_scalar_mul(w8, wf, W8SCALE)
                w8s.append(w8)
                nats.append(nat)

                # weighted sum lagged two chunks (software pipeline: gives the
                # softmax chain two chunk-periods before PE needs its w8)
                if c >= 2:
                    _wsum(nc, acc, nats[c - 2], w8s[c - 2], first=(c == 2), last=False)
            _wsum(nc, acc, nats[NCH - 2], w8s[NCH - 2], first=(NCH == 2), last=False)
            _wsum(nc, acc, nats[NCH - 1], w8s[NCH - 1], first=False, last=True)

            # ---- batch epilogue ----
            colsum = work.tile([128, 1], F32, tag="colsum")
            nc.vector.reduce_sum(out=colsum, in_=w_all, axis=mybir.AxisListType.X)
            tot = psum_e.tile([1, 1], F32, tag="eps")
            nc.tensor.matmul(tot, lhsT=ones_col, rhs=colsum, start=True, stop=True)
            inv = work.tile([1, 1], F32, tag="inv")
            nc.vector.reciprocal(inv, tot)
            invb_ps = psum_e.tile([128, 1], F32, tag="eps")
            nc.tensor.matmul(invb_ps, lhsT=ones_row, rhs=inv, start=True, stop=True)
            invb = work.tile([128, 1], F32, tag="invb")
            nc.vector.tensor_copy(invb, invb_ps)

            # attention output, [p, j] layout (host reorders)
            att_sb = wbuf.tile([128, NT], F32, tag="att_sb")
            nc.vector.tensor_scalar_mul(att_sb, w_all, invb)
            nc.sync.dma_start(out=atto[b], in_=att_sb)

            # accs_row = acc * inv / W8SCALE   [1, D]
            accs = work.tile([1, D], F32, tag="accs")
            nc.vector.tensor_scalar(
                accs,
                acc,
                scalar1=inv,
                scalar2=1.0 / W8SCALE,
                op0=mybir.AluOpType.mult,
                op1=mybir.AluOpType.mult,
            )
            # transpose accs halves to columns, then project: out = accsT @ WvT + bv
            accT = work.tile([128, 2], F32, tag="accT")
            for h in range(2):
                tp = psum_e.tile([128, 1], F32, tag="eps")
                nc.tensor.matmul(
                    tp,
                    lhsT=accs[:, h * 128 : (h + 1) * 128],
                    rhs=ones_row[0:1, 0:1],
                    start=True,
                    stop=True,
                )
                nc.vector.tensor_copy(accT[:, h : h + 1], tp)
            pout = psum_e.tile([1, D], F32, tag="eps")
            for h in range(2):
                nc.tensor.matmul(
                    pout,
                    lhsT=accT[:, h : h + 1],
                    rhs=wvt_sb[:, h, :],
                    start=(h == 0),
                    stop=(h == 1),
                )
            outv_sb = work.tile([1, D], F32, tag="outv_sb")
            nc.vector.tensor_add(outv_sb, pout, bv_sb)
            nc.sync.dma_start(out=outv[b : b + 1, :], in_=outv_sb)

    nc.finalize()
    return nc


def _prep(input_embedding, mask, Wv, bv, Wk, bk, queries):
    x = np.asarray(input_embedding, dtype=np.float32)
    mask = np.asarray(mask)
    Wv = np.asarray(Wv, dtype=np.float32)
    bv = np.asarray(bv, dtype=np.float32)
    Wk = np.asarray(Wk, dtype=np.float32)
    bk = np.asarray(bk, dtype=np.float32)
    queries = np.asarray(queries, dtype=np.float32)

    qt = (queries @ Wk) / np.sqrt(D).astype(np.float32)      # [N, D] = Q~
    c = (queries @ bk) / np.sqrt(D).astype(np.float32)       # [N]
    cscale = np.exp(c - c[IDX]).astype(np.float32)           # [N]

    # pretile: chunk c covers positions [c*CH, (c+1)*CH); within a chunk,
    # SBUF partition p / sub-tile t holds position c*CH + t*128 + p.
    # nat[b, c, p, t*D+d] = x[b, c*CH + t*128 + p, d]
    nat8 = np.ascontiguousarray(
        x.reshape(B, NCH, T, 128, D).transpose(0, 1, 3, 2, 4).reshape(B, NCH, 128, T * D)
    ).astype(np_bf16)
    # inT[b, c, p, h*CH+s] = x[b, c*CH + s, 128*h + p]
    xT = x.transpose(0, 2, 1)  # [B, D, S]
    inT8 = np.ascontiguousarray(
        xT.reshape(B, 2, 128, NCH, CH).transpose(0, 3, 2, 1, 4).reshape(B, NCH, 128, 2 * CH)
    ).astype(np_fp8)
    # msk[b, c, p, t*N+n] = mask[b, n, c*CH + t*128 + p] * cscale[n]
    mT = mask.transpose(0, 2, 1).astype(np.float32) * cscale[None, None, :]  # [B, S, N]
    mskT = np.ascontiguousarray(
        mT.reshape(B, NCH, T, 128, N).transpose(0, 1, 3, 2, 4).reshape(B, NCH, 128, T * N)
    ).astype(np_bf16)
    qt8 = np.ascontiguousarray((qt * Q8SCALE).T).astype(np_fp8)  # [D, N]
    wvt = np.ascontiguousarray(Wv.T)                         # [D, D]
    return nat8, inT8, mskT, qt8, wvt, bv.reshape(1, D)


_CACHE = {}


def _run(inputs, trace=False):
    nat8, inT8, mskT, qt8, wvt, bvr = _prep(**inputs)
    if "nc" not in _CACHE:
        _CACHE["nc"] = _build_nc()
    nc = _CACHE["nc"]
    in_maps = []
    for i in range(NCORES):
        sl = slice(i * BPC, (i + 1) * BPC)
        in_maps.append(
            {
                "nat": nat8[sl],
                "inT": inT8[sl],
                "msk": mskT[sl],
                "qt": qt8,
                "wvt": wvt,
                "bv": bvr,
            }
        )
    res = run_bass_kernel_spmd(nc, in_maps, core_ids=list(range(NCORES)), trace=trace)
    outs = []
    atts = []
    for i in range(NCORES):
        outs.append(np.asarray(res.results[i]["outv"], dtype=np.float32))
        a = np.asarray(res.results[i]["att"], dtype=np.float32)  # [BPC, 128, NT]
        atts.append(a.transpose(0, 2, 1).reshape(BPC, S))        # s = j*128 + p
    out = np.concatenate(outs, axis=0).reshape(B, 1, D)
    attention = np.concatenate(atts, axis=0).reshape(B, 1, S)
    return (out, attention), res


def kernel(**inputs):
    (out, attention), _ = _run(inputs, trace=False)
    return out, attention
